# revision 10
# baseline (speedup 1.0000x reference)
"""Trainium2 Bass kernel for nn_Adapter (LayerNorm -> 768->64 -> ReLU -> 64->768 -> *0.1).

Data-parallel across 8 NeuronCores: x (16,4096,768) flattens to 65536 tokens,
8192 tokens per core; the tiny adapter weights are replicated. No collectives.

Math (host folds the affine params into the weights):
    G      = gamma[:,None] * W_down                  (768,64)
    r1w    = [[c2],[-c1]] with c1 = gamma@W_down, c2 = beta@W_down + b_down
    W_up'  = SCALE * [W_up; b_up]                    (65,768)
Per token t (mu = mean, s = sqrt(var+eps), r = 1/s):
    P[.,t]   = G.T x[t] + s[t]*c2 - mu[t]*c1         (PSUM accumulate)
    z        = relu(P)            (r>0 lets the per-token scale move past relu)
    out[t,.] = r[t] * ( [z; s[t]] .T @ W_up' )       (r applied in PSUM->SBUF copy)
"""

from contextlib import ExitStack

import numpy as np

import concourse.bass as bass
import concourse.tile as tile
from concourse import bacc, mybir
from concourse.bass_utils import run_bass_kernel_spmd
from concourse.masks import make_identity

F32 = mybir.dt.float32
MM_DT = mybir.dt.float32r  # full-rate fp32 PE mode (N>=256)

P = 128            # tokens per tile (SBUF partitions)
D = 768            # model dim
R = 64             # bottleneck
NCHUNK = D // P    # 6 contraction chunks
TPB = 4            # token-tiles per block
BLK = P * TPB      # 512 tokens per block
N_CORES = 8
TOKENS = 16 * 4096
TOK_PER_CORE = TOKENS // N_CORES   # 8192
NBLK = TOK_PER_CORE // BLK         # 16
LN_EPS = 1e-5
SCALE = 0.1

_GRAPH_CACHE = {}


def _build_graph():
    nc = bacc.Bacc(
        "TRN2", target_bir_lowering=False, debug=False, num_devices=N_CORES
    )
    x_ext = nc.dram_tensor("x", [TOK_PER_CORE, D], F32, kind="ExternalInput").ap()
    g_ext = nc.dram_tensor("g", [D, R], F32, kind="ExternalInput").ap()
    r1_ext = nc.dram_tensor("r1w", [2, R], F32, kind="ExternalInput").ap()
    wup_ext = nc.dram_tensor("wup", [R + 1, D], F32, kind="ExternalInput").ap()
    out_ext = nc.dram_tensor("out", [TOK_PER_CORE, D], F32, kind="ExternalOutput").ap()

    with tile.TileContext(nc) as tc, ExitStack() as ctx:
        singles = ctx.enter_context(tc.tile_pool(name="singles", bufs=1))
        xpool = ctx.enter_context(tc.tile_pool(name="xp", bufs=2))
        xtpool = ctx.enter_context(tc.tile_pool(name="xtp", bufs=2))
        zpool = ctx.enter_context(tc.tile_pool(name="zp", bufs=2))
        opool = ctx.enter_context(tc.tile_pool(name="op", bufs=2))
        spool = ctx.enter_context(tc.tile_pool(name="sp", bufs=2))
        ps_t = ctx.enter_context(tc.tile_pool(name="ps_t", bufs=2, space="PSUM"))
        ps_p = ctx.enter_context(tc.tile_pool(name="ps_p", bufs=2, space="PSUM"))
        ps_up = ctx.enter_context(tc.tile_pool(name="ps_up", bufs=2, space="PSUM"))

        # one-time constants
        ident = singles.tile([P, P], F32)
        make_identity(nc, ident)
        # weights staged f32 then cast once to f32r (PE's rounded-fp32 format)
        gsb_f = singles.tile([P, NCHUNK, R], F32)
        nc.sync.dma_start(out=gsb_f, in_=g_ext.rearrange("(k p) r -> p k r", p=P))
        gsb = singles.tile([P, NCHUNK, R], MM_DT)
        nc.vector.tensor_copy(out=gsb, in_=gsb_f)
        wup_f = singles.tile([R + 1, D], F32)
        nc.sync.dma_start(out=wup_f, in_=wup_ext)
        wup = singles.tile([R + 1, D], MM_DT)
        nc.vector.tensor_copy(out=wup, in_=wup_f)
        r1w_f = singles.tile([R + 2, R], F32)   # rows 64..65 hold [c2; -c1]
        nc.sync.dma_start(out=r1w_f[R : R + 2, :], in_=r1_ext)
        r1w = singles.tile([R + 2, R], MM_DT)
        nc.vector.tensor_copy(out=r1w[R : R + 2, :], in_=r1w_f[R : R + 2, :])
        eps_t = singles.tile([P, 1], F32)
        nc.vector.memset(eps_t, LN_EPS)

        xv = x_ext.rearrange("(n i p) d -> n p i d", i=TPB, p=P)
        ov = out_ext.rearrange("(n i p) d -> n p i d", i=TPB, p=P)

        for b in range(NBLK):
            x_t = xpool.tile([P, TPB, D], F32)
            nc.sync.dma_start(out=x_t, in_=xv[b])

            xts = xtpool.tile([P, NCHUNK, BLK], MM_DT)
            z_aug = zpool.tile([R + 2, BLK], MM_DT)  # rows 0..63 relu(P), 64 s, 65 mu
            stats = spool.tile([P, TPB, 2, 6], F32)
            mv = spool.tile([P, TPB, 2], F32)
            st = spool.tile([P, TPB, 2], F32)      # col0 = s, col1 = mu
            rinv = spool.tile([P, TPB], F32)

            # per-token LN stats (DVE bn_stats on 512+256 chunks)
            for i in range(TPB):
                nc.vector.bn_stats(out=stats[:, i, 0, :], in_=x_t[:, i, 0:512])
                nc.vector.bn_stats(out=stats[:, i, 1, :], in_=x_t[:, i, 512:D])
                nc.vector.bn_aggr(out=mv[:, i, :], in_=stats[:, i, :, :])
            # batched tiny ops: s = sqrt(var+eps) (ACT), mu copy (DVE), 1/s (DVE)
            nc.scalar.activation(
                out=st[:, :, 0:1], in_=mv[:, :, 1:2],
                func=mybir.ActivationFunctionType.Sqrt, bias=eps_t,
            )
            nc.vector.tensor_copy(out=st[:, :, 1:2], in_=mv[:, :, 0:1])
            nc.vector.reciprocal(out=rinv, in_=st[:, :, 0:1])

            # per-tile stats row-transpose [128,2] -> [2,128] -> z_aug rows 64..65
            for i in range(TPB):
                mt_ps = ps_t.tile([2, P], F32, tag="tps")
                nc.tensor.transpose(mt_ps, st[:, i, :], ident)
                nc.vector.tensor_copy(
                    out=z_aug[R : R + 2, P * i : P * (i + 1)], in_=mt_ps
                )

            # x transposes: 4 tiles x 6 chunks; chunk k of all 4 tiles shares one
            # PSUM bank -> one wide [128, 512] drain (cast to f32r)
            for k in range(NCHUNK):
                t_ps = ps_t.tile([P, BLK], F32, tag="tps")
                for i in range(TPB):
                    nc.tensor.matmul(
                        t_ps[:, P * i : P * (i + 1)],
                        lhsT=x_t[:, i, P * k : P * (k + 1)],
                        rhs=ident,
                        is_transpose=True,
                        start=(i == 0),
                        stop=(i == TPB - 1),
                    )
                if k % 2 == 0:
                    nc.vector.tensor_copy(out=xts[:, k, :], in_=t_ps)
                else:
                    nc.scalar.copy(out=xts[:, k, :], in_=t_ps)

            # down-proj: P[64, 512] accumulates 6 K-chunks + rank-2 stats term
            p_ps = ps_p.tile([R, BLK], F32)
            for k in range(NCHUNK):
                nc.tensor.matmul(
                    p_ps,
                    lhsT=gsb[:, k, :],
                    rhs=xts[:, k, :],
                    start=(k == 0),
                    stop=False,
                )
            nc.tensor.matmul(
                p_ps,
                lhsT=r1w[R : R + 2, :],
                rhs=z_aug[R : R + 2, :],
                start=False,
                stop=True,
            )
            nc.scalar.activation(
                out=z_aug[0:R, :], in_=p_ps, func=mybir.ActivationFunctionType.Relu
            )

            # up-proj per 128-token tile + scaled PSUM->SBUF drain + store
            o_t = opool.tile([P, TPB, D], F32)
            for i in range(TPB):
                up_ps = ps_up.tile([P, D], F32)
                lhsT = z_aug[0 : R + 1, P * i : P * (i + 1)]
                nc.tensor.matmul(
                    up_ps[:, 0:512], lhsT=lhsT, rhs=wup[:, 0:512],
                    start=True, stop=True,
                )
                nc.tensor.matmul(
                    up_ps[:, 512:D], lhsT=lhsT, rhs=wup[:, 512:D],
                    start=True, stop=True,
                )
                sc = rinv[:, i : i + 1]
                if i == 0:
                    nc.vector.tensor_scalar_mul(out=o_t[:, i, :], in0=up_ps, scalar1=sc)
                else:
                    nc.scalar.mul(out=o_t[:, i, :], in_=up_ps, mul=sc)
            nc.sync.dma_start(out=ov[b], in_=o_t)

    nc.compile()
    return nc


def _get_graph():
    if "nc" not in _GRAPH_CACHE:
        _GRAPH_CACHE["nc"] = _build_graph()
    return _GRAPH_CACHE["nc"]


def kernel(x, ln_gamma, ln_beta, W_down, b_down, W_up, b_up, **kw):
    x = np.asarray(x, dtype=np.float32)
    ln_gamma = np.asarray(ln_gamma, dtype=np.float32)
    ln_beta = np.asarray(ln_beta, dtype=np.float32)
    W_down = np.asarray(W_down, dtype=np.float32)
    b_down = np.asarray(b_down, dtype=np.float32)
    W_up = np.asarray(W_up, dtype=np.float32)
    b_up = np.asarray(b_up, dtype=np.float32)

    orig_shape = x.shape
    xf = np.ascontiguousarray(x.reshape(TOKENS, D))

    # host-side weight folding (tiny)
    g = np.ascontiguousarray(ln_gamma[:, None] * W_down)              # (768,64)
    c1 = ln_gamma @ W_down                                            # (64,)
    c2 = ln_beta @ W_down + b_down                                    # (64,)
    r1w = np.ascontiguousarray(np.stack([c2, -c1]).astype(np.float32))  # (2,64)
    wup = np.ascontiguousarray(
        (SCALE * np.concatenate([W_up, b_up[None, :]], axis=0)).astype(np.float32)
    )                                                                 # (65,768)

    nc = _get_graph()
    in_maps = [
        {
            "x": np.ascontiguousarray(xf[i * TOK_PER_CORE : (i + 1) * TOK_PER_CORE]),
            "g": g,
            "r1w": r1w,
            "wup": wup,
        }
        for i in range(N_CORES)
    ]
    res = run_bass_kernel_spmd(nc, in_maps, core_ids=list(range(N_CORES)))
    out = np.concatenate([res.results[i]["out"] for i in range(N_CORES)], axis=0)
    return out.reshape(orig_shape)
